# revision 11
# baseline (speedup 1.0000x reference)
"""Trainium2 Bass kernel: batched causal single-head self-attention.

Reference computation (per batch b):
    q = x @ Wq; k = x @ Wk; v = x @ Wv          # [T, H] each, contraction over E
    S = (q @ k^T) / sqrt(H)                     # [T, T]
    P = softmax(causal_mask(S), axis=-1)
    out = P @ v                                 # [T, H]

Shapes: x [512, 256, 384] f32, W* [384, 64] f32, out [512, 256, 64] f32.
Sharding: pure data parallel, 64 batches per NeuronCore across 8 cores.

Device algorithm (all matmul operands bf16, fp32 PSUM):
  - host ships xt[p, b, c, t] = x[b, t, 128c+p] so every projection matmul
    has its E-contraction on partitions and per-partition DMA runs are 6KB.
  - q/k projections col-tiled per batch pair: tile(0,0) computes batch s0,
    tile(0,64) computes s1 concurrently -> qk_ps[0:64]=s0 {q,k},
    [64:128]=s1 {q,k}.  Both q and k of a batch land on the SAME partition
    half, so the score matmuls read them in place (no partition shifts).
  - v projection col-tiled across the two quads of a block: tile(0,0) is
    quadA, tile(0,64) quadB -> v^T for 8 batches in half the matmul time.
  - v^T -> v via the DMA xbar transpose (dma_start_transpose), writing
    straight into an SBUF tile with a persistent ones column at h=64
    (gives softmax denominators through the out matmul for free).
  - scores row-tiled per pair: tile rows 0:64 = s0, rows 64:128 = s1 run
    concurrently (contraction is only H=64).  S^T layout [tk, tq]; the
    lower-left T/4 block is skipped (causal).
  - P = exp(0.125*S^T) on ScalarE; multiplicative tril mask on the two
    diagonal blocks runs on the otherwise-idle GpSimd engine.
  - out_aug[tq, 0:65] = sum_tk P[tk,tq]*[v|1][tk]; col 64 is the softmax
    denominator.  The division happens on the HOST (f32) - the device
    ships bf16 numerators+denominators.
  - emission order is software-pipelined with a 2-pair lag so the tensor
    queue never stalls on ACT/DVE/GpSimd results.
"""

import numpy as np
import ml_dtypes

B, T, E, H = 512, 256, 384, 64
NCORES = 8
BPC = B // NCORES  # 64
P = 128
EC = E // P  # 3
HP1 = H + 1  # 65
Q = 4  # batches per quad

_cache: dict = {}


def _install_ntff_hook():
    """Shim antenv.axon_hooks (absent in this image) so run_bass_kernel_spmd
    trace=True can capture NTFF profiles via the axon .so's C ABI."""
    import contextlib
    import ctypes
    import sys
    import types

    if "antenv.axon_hooks" in sys.modules:
        return
    so_path = "/opt/axon/libaxon_pjrt.so"
    lib = ctypes.CDLL(so_path)
    if not hasattr(lib, "axon_start_nrt_profile"):
        return
    lib.axon_start_nrt_profile.argtypes = [
        ctypes.POINTER(ctypes.c_int64),
        ctypes.c_size_t,
    ]
    lib.axon_start_nrt_profile.restype = ctypes.c_int64
    lib.axon_stop_nrt_profile.argtypes = [ctypes.c_char_p]
    lib.axon_stop_nrt_profile.restype = ctypes.c_int64

    @contextlib.contextmanager
    def _hook(output_dir, device_ids):
        import jax

        jax.devices()
        if device_ids:
            ids = (ctypes.c_int64 * len(device_ids))(*device_ids)
            rc = lib.axon_start_nrt_profile(ids, len(device_ids))
        else:
            rc = lib.axon_start_nrt_profile(None, 0)
        if rc != 0:
            raise RuntimeError(f"axon_start_nrt_profile rc={rc}")
        try:
            yield
        finally:
            n = lib.axon_stop_nrt_profile(str(output_dir).encode())
            if n < 0:
                raise RuntimeError(f"axon_stop_nrt_profile rc={n}")
            print(f"profile: {n} file(s) written to {output_dir}", file=sys.stderr)

    mod = types.ModuleType("antenv.axon_hooks")
    _state = {"hook": _hook}
    mod.get_axon_ntff_profile_hook = lambda: _state["hook"]
    mod.set_axon_ntff_profile_hook = lambda h: _state.__setitem__("hook", h)
    sys.modules["antenv.axon_hooks"] = mod


def _build_program(bpc):
    import concourse.bacc as bacc
    import concourse.mybir as mybir
    import concourse.tile as tile

    f32 = mybir.dt.float32
    bf16 = mybir.dt.bfloat16
    Exp = mybir.ActivationFunctionType.Exp
    Mult = mybir.AluOpType.mult

    nc = bacc.Bacc(
        "TRN2",
        target_bir_lowering=False,
        debug=False,
        enable_asserts=False,
        num_devices=NCORES,
    )
    nq = bpc // Q  # quads
    npair = bpc // 2  # batch pairs
    assert bpc % Q == 0

    xt_d = nc.dram_tensor("xt", [P, bpc, EC, T], bf16, kind="ExternalInput").ap()
    wqk_d = nc.dram_tensor("wqk", [P, EC, P], bf16, kind="ExternalInput").ap()
    wv_d = nc.dram_tensor("wv", [P, EC, H], bf16, kind="ExternalInput").ap()
    um_d = nc.dram_tensor("um", [P, P], bf16, kind="ExternalInput").ap()
    out_d = nc.dram_tensor("out", [nq, P, Q, 2, HP1], bf16, kind="ExternalOutput").ap()

    with tile.TileContext(nc) as tc:
        with (
            tc.tile_pool(name="const", bufs=1) as constp,
            tc.tile_pool(name="xin", bufs=5) as xpool,
            tc.tile_pool(name="qksb", bufs=3) as qkpool,
            tc.tile_pool(name="scrp", bufs=3) as scrpool,
            tc.tile_pool(name="vtb", bufs=2) as vtpool,
            tc.tile_pool(name="vrw", bufs=2) as vrpool,
            tc.tile_pool(name="psb", bufs=3) as ppool,
            tc.tile_pool(name="osb", bufs=2) as opool,
            tc.tile_pool(name="ps_qk", bufs=2, space="PSUM") as ps_qk,
            tc.tile_pool(name="ps_v", bufs=1, space="PSUM") as ps_v,
            tc.tile_pool(name="ps_s", bufs=2, space="PSUM") as ps_s,
            tc.tile_pool(name="ps_o", bufs=1, space="PSUM") as ps_o,
        ):
            wqk = constp.tile([P, EC, P], bf16)
            nc.sync.dma_start(wqk, wqk_d)
            wv = constp.tile([P, EC, H], bf16)
            nc.sync.dma_start(wv, wv_d)
            um = constp.tile([P, P], bf16)
            nc.sync.dma_start(um, um_d)
            # v staging ring with persistent ones column at h=64 (softmax
            # denominators come out of the out-matmul for free)
            vsbs = []
            for r in range(3):
                vsb = constp.tile([P, 2 * Q, HP1], bf16, name=f"vsb{r}")
                nc.vector.memset(vsb[:, :, H:HP1], 1.0)
                vsbs.append(vsb)

            xts = {}
            qk_sbs = {}
            scrs = {}
            p_sbs = {}
            o_sbs = {}
            o_pss = {}

            def fetch(q):
                if q < nq:
                    xt = xpool.tile([P, Q, EC, T], bf16, name="xt")
                    nc.sync.dma_start(xt, xt_d[:, Q * q : Q * q + Q])
                    xts[q] = xt

            def quad_head(q):
                """xt prefetch, v projection (t-half col-tiled), v^T cast,
                dense DMA transpose, strided copy into the ones-col ring."""
                fetch(q + 4)
                xt = xts[q]
                v_ps = ps_v.tile([P, Q, P], f32, name="v_ps")
                # group-major: each PSUM accumulation group's matmuls stay
                # consecutive (interleaving groups corrupts earlier groups)
                for sh in range(2):
                    s0 = 2 * sh
                    for base in (0, 1):
                        for c in range(EC):
                            nc.tensor.matmul(
                                v_ps[64 * base : 64 * base + 64, s0 : s0 + 2, :],
                                wv[:, c, :],
                                xt[:, s0 : s0 + 2, c, 128 * base : 128 * base + 128],
                                start=(c == 0),
                                stop=(c == EC - 1),
                            )
                vtab = vtpool.tile([P, Q, P], bf16, name="vtab")
                nc.vector.tensor_copy(vtab, v_ps)
                vraw = vrpool.tile([P, Q, P], bf16, name="vraw")
                # vraw[p, s, 64j+h] = v[t=128j+p, s-batch, h]
                nc.sync.dma_start_transpose(
                    vraw, vtab.rearrange("p s t -> p (s t)")
                )
                vsb = vsbs[q % 3]
                nc.gpsimd.tensor_copy(
                    vsb.rearrange("p (s j) h -> p s j h", j=2)[:, :, :, 0:H],
                    vraw.rearrange("p s (j h) -> p s j h", j=2),
                )
                return vsb

            def qk_stage(i):
                """packed [Wq|Wk] projection for pair i (full 128-wide)."""
                q = i // 2
                xt = xts[q]
                s0 = 2 * (i % 2)
                qk_ps = ps_qk.tile([P, 2, T], f32, name="qk_ps")
                for c in range(EC):
                    nc.tensor.matmul(
                        qk_ps,
                        wqk[:, c, :],
                        xt[:, s0 : s0 + 2, c, :],
                        start=(c == 0),
                        stop=(c == EC - 1),
                    )
                if i % 2 == 0:
                    qk_sbs[q] = qkpool.tile([P, Q, T], bf16, name="qk_sb")
                qk_sb = qk_sbs[q]
                nc.vector.tensor_copy(qk_sb[:, s0 : s0 + 2, :], qk_ps)
                if i % 2 == 1:
                    # partition shifts so both q and k of a batch sit on one
                    # half: even s -> k to rows 0:64, odd s -> q to rows 64:128
                    scr = scrpool.tile([P, Q, T], bf16, name="scr")
                    qv = qk_sb.rearrange("p (a b) t -> p a b t", b=2)
                    sv = scr.rearrange("p (a b) t -> p a b t", b=2)
                    nc.sync.dma_start(sv[0:H, :, 0, :], qv[H:P, :, 0, :])
                    nc.sync.dma_start(sv[H:P, :, 1, :], qv[0:H, :, 1, :])
                    scrs[q] = scr

            def score_stage(i):
                """row-tiled scores (s0 on rows 0:64 | s1 on rows 64:128),
                one exp per pair, tril mask on diagonal blocks (DVE)."""
                q = i // 2
                qk_sb, scr = qk_sbs[q], scrs[q]
                s0 = 2 * (i % 2)
                s1 = s0 + 1
                s_ps = ps_s.tile([P, 2, 2 * T], f32, name="s_ps")
                nc.tensor.matmul(
                    s_ps[:, 0, 0:T], scr[0:H, s0, 0:P], qk_sb[0:H, s0, :],
                    start=True, stop=True,
                )
                nc.tensor.matmul(
                    s_ps[:, 0, T : T + P], scr[0:H, s0, P:T], qk_sb[0:H, s0, P:T],
                    start=True, stop=True,
                )
                nc.tensor.matmul(
                    s_ps[:, 1, 0:T], qk_sb[H:P, s1, 0:P], scr[H:P, s1, :],
                    start=True, stop=True,
                )
                nc.tensor.matmul(
                    s_ps[:, 1, T : T + P], qk_sb[H:P, s1, P:T], scr[H:P, s1, P:T],
                    start=True, stop=True,
                )
                p_sb = ppool.tile([P, 2, 3 * P], bf16, name="p_sb")
                nc.scalar.activation(
                    p_sb, s_ps[:, :, 0 : 3 * P], Exp, scale=0.125
                )
                pv = p_sb.rearrange("p s (b x) -> p s b x", x=P)[:, :, ::2, :]
                nc.vector.tensor_tensor(
                    pv, pv, um[:, None, None, :].to_broadcast([P, 2, 2, P]), Mult
                )
                p_sbs[i] = p_sb

            def out_stage(i):
                """out_aug = P^T-contracted [v|1] for pair i."""
                p_sb = p_sbs.pop(i)
                q = i // 2
                vsb = vsbs[q % 3]
                o_ps = ps_o.tile([P, 2, 2, HP1], f32, name="o_ps")
                for s in range(2):
                    e0 = 2 * (2 * (i % 2) + s)
                    nc.tensor.matmul(
                        o_ps[:, s, 0, :], p_sb[:, s, 0:P], vsb[:, e0, :],
                        start=True, stop=True,
                    )
                    nc.tensor.matmul(
                        o_ps[:, s, 1, :], p_sb[:, s, P:T], vsb[:, e0, :],
                        start=True, stop=False,
                    )
                    nc.tensor.matmul(
                        o_ps[:, s, 1, :], p_sb[:, s, T : 3 * P], vsb[:, e0 + 1, :],
                        start=False, stop=True,
                    )
                if i % 2 == 0:
                    o_sbs[q] = opool.tile([P, Q, 2, HP1], bf16, name="o_sb")
                nc.scalar.copy(
                    o_sbs[q][:, 2 * (i % 2) : 2 * (i % 2) + 2], o_ps
                )
                if i % 2 == 1:
                    nc.sync.dma_start(out_d[q], o_sbs.pop(q))

            for q in range(4):
                fetch(q)
            for i in range(npair + 4):
                if i < npair and i % 2 == 0:
                    quad_head(i // 2)
                if i < npair:
                    qk_stage(i)
                if 2 <= i < npair + 2:
                    score_stage(i - 2)
                if i >= 4:
                    out_stage(i - 4)

    nc.compile()
    return nc


def _prep_inputs(x, Wq, Wk, Wv, bpc):
    bf = ml_dtypes.bfloat16
    nb = NCORES * bpc
    x = np.asarray(x, dtype=np.float32)[:nb]
    # [b, t, e] -> [p, b, c, t] with e = c*128 + p
    xt = np.ascontiguousarray(
        x.reshape(nb, T, EC, P).transpose(3, 0, 2, 1)
    ).astype(bf)
    def wprep(w):
        return np.ascontiguousarray(
            np.asarray(w, np.float32).reshape(EC, P, H).transpose(1, 0, 2)
        ).astype(bf)
    um = (np.arange(P)[:, None] <= np.arange(P)[None, :]).astype(np.float32).astype(bf)
    wqk = np.concatenate(
        [np.asarray(Wq, np.float32), np.asarray(Wk, np.float32)], axis=1
    )  # [E, 128]
    wqkp = np.ascontiguousarray(
        wqk.reshape(EC, P, P).transpose(1, 0, 2)
    ).astype(bf)
    wvp = wprep(Wv)
    per_core = []
    for c in range(NCORES):
        per_core.append(
            {
                "xt": np.ascontiguousarray(xt[:, c * bpc : (c + 1) * bpc]),
                "wqk": wqkp,
                "wv": wvp,
                "um": um,
            }
        )
    return per_core


def kernel(x, Wq, Wk, Wv, _trace=False, _bpc=BPC):
    """Full inputs in, full output out. Shards batch dim over 8 NeuronCores."""
    from concourse import bass_utils

    if _trace:
        _install_ntff_hook()

    key = ("prog", _bpc)
    if key not in _cache:
        _cache[key] = _build_program(_bpc)
    nc = _cache[key]

    in_maps = _prep_inputs(x, Wq, Wk, Wv, _bpc)
    res = bass_utils.run_bass_kernel_spmd(
        nc, in_maps, core_ids=list(range(NCORES)), trace=_trace
    )
    _cache["last_result"] = res
    outs = []
    for r in res.results:
        o = np.asarray(r["out"], dtype=np.float32)  # [nq, P, Q, 2, HP1]
        num, den = o[..., :H], o[..., H:]
        oc = num / den  # [nq, P, Q, 2, H]
        # b = quad*4 + s, t = j*128 + p
        oc = oc.transpose(0, 2, 3, 1, 4).reshape(_bpc, T, H)
        outs.append(oc)
    return np.concatenate(outs, axis=0).astype(np.float32)


# revision 12
# speedup vs baseline: 1.1616x; 1.1616x over previous
"""Trainium2 Bass kernel: batched causal single-head self-attention.

Reference computation (per batch b):
    q = x @ Wq; k = x @ Wk; v = x @ Wv          # [T, H] each, contraction over E
    S = (q @ k^T) / sqrt(H)                     # [T, T]
    P = softmax(causal_mask(S), axis=-1)
    out = P @ v                                 # [T, H]

Shapes: x [512, 256, 384] f32, W* [384, 64] f32, out [512, 256, 64] f32.
Sharding: pure data parallel, 64 batches per NeuronCore across 8 cores.

Device algorithm (all matmul operands bf16, fp32 PSUM):
  - host ships xt[p, b, c, t] = x[b, t, 128c+p] so every projection matmul
    has its E-contraction on partitions and per-partition DMA runs are 6KB.
  - q/k projections col-tiled per batch pair: tile(0,0) computes batch s0,
    tile(0,64) computes s1 concurrently -> qk_ps[0:64]=s0 {q,k},
    [64:128]=s1 {q,k}.  Both q and k of a batch land on the SAME partition
    half, so the score matmuls read them in place (no partition shifts).
  - v projection col-tiled across the two quads of a block: tile(0,0) is
    quadA, tile(0,64) quadB -> v^T for 8 batches in half the matmul time.
  - v^T -> v via the DMA xbar transpose (dma_start_transpose), writing
    straight into an SBUF tile with a persistent ones column at h=64
    (gives softmax denominators through the out matmul for free).
  - scores row-tiled per pair: tile rows 0:64 = s0, rows 64:128 = s1 run
    concurrently (contraction is only H=64).  S^T layout [tk, tq]; the
    lower-left T/4 block is skipped (causal).
  - P = exp(0.125*S^T) on ScalarE; multiplicative tril mask on the two
    diagonal blocks runs on the otherwise-idle GpSimd engine.
  - out_aug[tq, 0:65] = sum_tk P[tk,tq]*[v|1][tk]; col 64 is the softmax
    denominator.  The division happens on the HOST (f32) - the device
    ships bf16 numerators+denominators.
  - emission order is software-pipelined with a 2-pair lag so the tensor
    queue never stalls on ACT/DVE/GpSimd results.
"""

import numpy as np
import ml_dtypes

B, T, E, H = 512, 256, 384, 64
NCORES = 8
BPC = B // NCORES  # 64
P = 128
EC = E // P  # 3
HP1 = H + 1  # 65
Q = 4  # batches per quad

_cache: dict = {}


def _install_ntff_hook():
    """Shim antenv.axon_hooks (absent in this image) so run_bass_kernel_spmd
    trace=True can capture NTFF profiles via the axon .so's C ABI."""
    import contextlib
    import ctypes
    import sys
    import types

    if "antenv.axon_hooks" in sys.modules:
        return
    so_path = "/opt/axon/libaxon_pjrt.so"
    lib = ctypes.CDLL(so_path)
    if not hasattr(lib, "axon_start_nrt_profile"):
        return
    lib.axon_start_nrt_profile.argtypes = [
        ctypes.POINTER(ctypes.c_int64),
        ctypes.c_size_t,
    ]
    lib.axon_start_nrt_profile.restype = ctypes.c_int64
    lib.axon_stop_nrt_profile.argtypes = [ctypes.c_char_p]
    lib.axon_stop_nrt_profile.restype = ctypes.c_int64

    @contextlib.contextmanager
    def _hook(output_dir, device_ids):
        import jax

        jax.devices()
        if device_ids:
            ids = (ctypes.c_int64 * len(device_ids))(*device_ids)
            rc = lib.axon_start_nrt_profile(ids, len(device_ids))
        else:
            rc = lib.axon_start_nrt_profile(None, 0)
        if rc != 0:
            raise RuntimeError(f"axon_start_nrt_profile rc={rc}")
        try:
            yield
        finally:
            n = lib.axon_stop_nrt_profile(str(output_dir).encode())
            if n < 0:
                raise RuntimeError(f"axon_stop_nrt_profile rc={n}")
            print(f"profile: {n} file(s) written to {output_dir}", file=sys.stderr)

    mod = types.ModuleType("antenv.axon_hooks")
    _state = {"hook": _hook}
    mod.get_axon_ntff_profile_hook = lambda: _state["hook"]
    mod.set_axon_ntff_profile_hook = lambda h: _state.__setitem__("hook", h)
    sys.modules["antenv.axon_hooks"] = mod


def _build_program(bpc):
    import concourse.bacc as bacc
    import concourse.mybir as mybir
    import concourse.tile as tile

    f32 = mybir.dt.float32
    bf16 = mybir.dt.bfloat16
    Exp = mybir.ActivationFunctionType.Exp
    Mult = mybir.AluOpType.mult

    nc = bacc.Bacc(
        "TRN2",
        target_bir_lowering=False,
        debug=False,
        enable_asserts=False,
        num_devices=NCORES,
    )
    nq = bpc // Q  # quads
    npair = bpc // 2  # batch pairs
    assert bpc % Q == 0

    xt_d = nc.dram_tensor("xt", [P, bpc, EC, T], bf16, kind="ExternalInput").ap()
    wqk_d = nc.dram_tensor("wqk", [P, EC, P], bf16, kind="ExternalInput").ap()
    wv_d = nc.dram_tensor("wv", [P, EC, H], bf16, kind="ExternalInput").ap()
    um_d = nc.dram_tensor("um", [P, P], bf16, kind="ExternalInput").ap()
    out_d = nc.dram_tensor("out", [nq, P, Q, 2, HP1], bf16, kind="ExternalOutput").ap()

    with tile.TileContext(nc) as tc:
        with (
            tc.tile_pool(name="const", bufs=1) as constp,
            tc.tile_pool(name="xin", bufs=5) as xpool,
            tc.tile_pool(name="qksb", bufs=3) as qkpool,
            tc.tile_pool(name="scrp", bufs=3) as scrpool,
            tc.tile_pool(name="vtb", bufs=2) as vtpool,
            tc.tile_pool(name="vrw", bufs=2) as vrpool,
            tc.tile_pool(name="psb", bufs=3) as ppool,
            tc.tile_pool(name="osb", bufs=2) as opool,
            tc.tile_pool(name="ps_qk", bufs=2, space="PSUM") as ps_qk,
            tc.tile_pool(name="ps_v", bufs=1, space="PSUM") as ps_v,
            tc.tile_pool(name="ps_s", bufs=2, space="PSUM") as ps_s,
            tc.tile_pool(name="ps_o", bufs=1, space="PSUM") as ps_o,
        ):
            wqk = constp.tile([P, EC, P], bf16)
            nc.sync.dma_start(wqk, wqk_d)
            wv = constp.tile([P, EC, H], bf16)
            nc.sync.dma_start(wv, wv_d)
            um = constp.tile([P, P], bf16)
            nc.sync.dma_start(um, um_d)
            # v staging ring with persistent ones column at h=64 (softmax
            # denominators come out of the out-matmul for free)
            vsbs = []
            for r in range(3):
                vsb = constp.tile([P, 2 * Q, HP1], bf16, name=f"vsb{r}")
                nc.vector.memset(vsb[:, :, H:HP1], 1.0)
                vsbs.append(vsb)

            xts = {}
            qk_sbs = {}
            scrs = {}
            p_sbs = {}
            o_sbs = {}
            o_pss = {}

            def fetch(q):
                if q < nq:
                    xt = xpool.tile([P, Q, EC, T], bf16, name="xt")
                    nc.sync.dma_start(xt, xt_d[:, Q * q : Q * q + Q])
                    xts[q] = xt

            def quad_head(q):
                """xt prefetch, v projection (t-half col-tiled), v^T cast,
                dense DMA transpose, strided copy into the ones-col ring."""
                fetch(q + 4)
                xt = xts[q]
                v_ps = ps_v.tile([P, Q, P], f32, name="v_ps")
                # group-major: each PSUM accumulation group's matmuls stay
                # consecutive (interleaving groups corrupts earlier groups)
                for sh in range(2):
                    s0 = 2 * sh
                    for base in (0, 1):
                        for c in range(EC):
                            nc.tensor.matmul(
                                v_ps[64 * base : 64 * base + 64, s0 : s0 + 2, :],
                                wv[:, c, :],
                                xt[:, s0 : s0 + 2, c, 128 * base : 128 * base + 128],
                                start=(c == 0),
                                stop=(c == EC - 1),
                            )
                vtab = vtpool.tile([P, Q, P], bf16, name="vtab")
                nc.vector.tensor_copy(vtab, v_ps)
                vraw = vrpool.tile([P, Q, P], bf16, name="vraw")
                # vraw[p, s, 64j+h] = v[t=128j+p, s-batch, h]
                nc.sync.dma_start_transpose(
                    vraw, vtab.rearrange("p s t -> p (s t)")
                )
                vsb = vsbs[q % 3]
                nc.gpsimd.tensor_copy(
                    vsb.rearrange("p (s j) h -> p s j h", j=2)[:, :, :, 0:H],
                    vraw.rearrange("p s (j h) -> p s j h", j=2),
                )
                return vsb

            def qk_stage(i):
                """packed [Wq|Wk] projection for pair i (full 128-wide)."""
                q = i // 2
                xt = xts[q]
                s0 = 2 * (i % 2)
                qk_ps = ps_qk.tile([P, 2, T], f32, name="qk_ps")
                for c in range(EC):
                    nc.tensor.matmul(
                        qk_ps,
                        wqk[:, c, :],
                        xt[:, s0 : s0 + 2, c, :],
                        start=(c == 0),
                        stop=(c == EC - 1),
                    )
                if i % 2 == 0:
                    qk_sbs[q] = qkpool.tile([P, Q, T], bf16, name="qk_sb")
                qk_sb = qk_sbs[q]
                nc.vector.tensor_copy(qk_sb[:, s0 : s0 + 2, :], qk_ps)
                if i % 2 == 1:
                    # partition shifts so both q and k of a batch sit on one
                    # half: even s -> k to rows 0:64, odd s -> q to rows 64:128
                    scr = scrpool.tile([P, Q, T], bf16, name="scr")
                    qv = qk_sb.rearrange("p (a b) t -> p a b t", b=2)
                    sv = scr.rearrange("p (a b) t -> p a b t", b=2)
                    nc.gpsimd.dma_start(sv[0:H, :, 0, :], qv[H:P, :, 0, :])
                    nc.gpsimd.dma_start(sv[H:P, :, 1, :], qv[0:H, :, 1, :])
                    scrs[q] = scr

            def score_stage(i):
                """row-tiled scores (s0 on rows 0:64 | s1 on rows 64:128),
                one exp per pair, tril mask on diagonal blocks (DVE)."""
                q = i // 2
                qk_sb, scr = qk_sbs[q], scrs[q]
                s0 = 2 * (i % 2)
                s1 = s0 + 1
                s_ps = ps_s.tile([P, 2, 2 * T], f32, name="s_ps")
                nc.tensor.matmul(
                    s_ps[:, 0, 0:T], scr[0:H, s0, 0:P], qk_sb[0:H, s0, :],
                    start=True, stop=True,
                )
                nc.tensor.matmul(
                    s_ps[:, 0, T : T + P], scr[0:H, s0, P:T], qk_sb[0:H, s0, P:T],
                    start=True, stop=True,
                )
                nc.tensor.matmul(
                    s_ps[:, 1, 0:T], qk_sb[H:P, s1, 0:P], scr[H:P, s1, :],
                    start=True, stop=True,
                )
                nc.tensor.matmul(
                    s_ps[:, 1, T : T + P], qk_sb[H:P, s1, P:T], scr[H:P, s1, P:T],
                    start=True, stop=True,
                )
                p_sb = ppool.tile([P, 2, 3 * P], bf16, name="p_sb")
                nc.scalar.activation(
                    p_sb, s_ps[:, :, 0 : 3 * P], Exp, scale=0.125
                )
                pv = p_sb.rearrange("p s (b x) -> p s b x", x=P)[:, :, ::2, :]
                nc.vector.tensor_tensor(
                    pv, pv, um[:, None, None, :].to_broadcast([P, 2, 2, P]), Mult
                )
                p_sbs[i] = p_sb

            def out_stage(i):
                """out_aug = P^T-contracted [v|1] for pair i."""
                p_sb = p_sbs.pop(i)
                q = i // 2
                vsb = vsbs[q % 3]
                o_ps = ps_o.tile([P, 2, 2, HP1], f32, name="o_ps")
                for s in range(2):
                    e0 = 2 * (2 * (i % 2) + s)
                    nc.tensor.matmul(
                        o_ps[:, s, 0, :], p_sb[:, s, 0:P], vsb[:, e0, :],
                        start=True, stop=True,
                    )
                    nc.tensor.matmul(
                        o_ps[:, s, 1, :], p_sb[:, s, P:T], vsb[:, e0, :],
                        start=True, stop=False,
                    )
                    nc.tensor.matmul(
                        o_ps[:, s, 1, :], p_sb[:, s, T : 3 * P], vsb[:, e0 + 1, :],
                        start=False, stop=True,
                    )
                if i % 2 == 0:
                    o_sbs[q] = opool.tile([P, Q, 2, HP1], bf16, name="o_sb")
                nc.scalar.copy(
                    o_sbs[q][:, 2 * (i % 2) : 2 * (i % 2) + 2], o_ps
                )
                if i % 2 == 1:
                    nc.sync.dma_start(out_d[q], o_sbs.pop(q))

            for q in range(4):
                fetch(q)
            for i in range(npair + 4):
                if i < npair and i % 2 == 0:
                    quad_head(i // 2)
                if i < npair:
                    qk_stage(i)
                if 2 <= i < npair + 2:
                    score_stage(i - 2)
                if i >= 4:
                    out_stage(i - 4)

    nc.compile()
    return nc


def _prep_inputs(x, Wq, Wk, Wv, bpc):
    bf = ml_dtypes.bfloat16
    nb = NCORES * bpc
    x = np.asarray(x, dtype=np.float32)[:nb]
    # [b, t, e] -> [p, b, c, t] with e = c*128 + p
    xt = np.ascontiguousarray(
        x.reshape(nb, T, EC, P).transpose(3, 0, 2, 1)
    ).astype(bf)
    def wprep(w):
        return np.ascontiguousarray(
            np.asarray(w, np.float32).reshape(EC, P, H).transpose(1, 0, 2)
        ).astype(bf)
    um = (np.arange(P)[:, None] <= np.arange(P)[None, :]).astype(np.float32).astype(bf)
    wqk = np.concatenate(
        [np.asarray(Wq, np.float32), np.asarray(Wk, np.float32)], axis=1
    )  # [E, 128]
    wqkp = np.ascontiguousarray(
        wqk.reshape(EC, P, P).transpose(1, 0, 2)
    ).astype(bf)
    wvp = wprep(Wv)
    per_core = []
    for c in range(NCORES):
        per_core.append(
            {
                "xt": np.ascontiguousarray(xt[:, c * bpc : (c + 1) * bpc]),
                "wqk": wqkp,
                "wv": wvp,
                "um": um,
            }
        )
    return per_core


def kernel(x, Wq, Wk, Wv, _trace=False, _bpc=BPC):
    """Full inputs in, full output out. Shards batch dim over 8 NeuronCores."""
    from concourse import bass_utils

    if _trace:
        _install_ntff_hook()

    key = ("prog", _bpc)
    if key not in _cache:
        _cache[key] = _build_program(_bpc)
    nc = _cache[key]

    in_maps = _prep_inputs(x, Wq, Wk, Wv, _bpc)
    res = bass_utils.run_bass_kernel_spmd(
        nc, in_maps, core_ids=list(range(NCORES)), trace=_trace
    )
    _cache["last_result"] = res
    outs = []
    for r in res.results:
        o = np.asarray(r["out"], dtype=np.float32)  # [nq, P, Q, 2, HP1]
        num, den = o[..., :H], o[..., H:]
        oc = num / den  # [nq, P, Q, 2, H]
        # b = quad*4 + s, t = j*128 + p
        oc = oc.transpose(0, 2, 3, 1, 4).reshape(_bpc, T, H)
        outs.append(oc)
    return np.concatenate(outs, axis=0).astype(np.float32)
